# revision 3
# baseline (speedup 1.0000x reference)
"""MoE layer (top-2 of 8 experts) on 8 Trainium2 NeuronCores.

Strategy (expert-parallel along the *F axis* — "global F-split"):
  * Host computes the (tiny) gating network: probs = softmax(x @ w_gate),
    top-2 experts + normalized gates per token.
  * The expert FFN decomposes along the hidden axis F:
        o = relu(x @ W1) @ W2 = sum_fslices relu(x @ W1[:, fs]) @ W2[fs, :]
    so core c is given the f-slice [c*F/8, (c+1)*F/8) of EVERY expert's
    W1/W2 (16.8 MB bf16 — same footprint as one whole expert) and computes
    the partial output of EVERY assignment over its slice.  Per-core work
    is exactly sum_e load_e * F/8 = N*K*F/8 — perfectly balanced across
    cores regardless of expert load skew, with zero dropped tokens.
  * All cores run the SAME program on the SAME dispatched-token stream
    (assignments grouped by expert); only the weight slices differ.
  * Host combines: o = sum_cores o_partial;  y[n] = sum_k gate[n,k]*o[slot].

Device kernel layout (per core, SPMD over 8 cores):
  inputs  xT [D, A]    bf16  dispatched tokens, transposed, expert-grouped
          w1 [E, D, FS] bf16  this core's f-slice of every expert's W1
          w2 [E, FS, D] bf16  this core's f-slice of every expert's W2
  output  out [D, A]   bf16  partial expert outputs (transposed; host
                             untransposes during the combine)
  Tokens are processed in strips of up to TS=512 (any remainder exact —
  no padding anywhere, both matmuls scale with the true token count):
    mm1: ph[f, t]  = sum_ki w1[ki,f].T @ xT[ki, t]   (f on PSUM partitions)
    relu -> h bf16 [f, 4fc, t]                        (scalar engine)
    mm2: po[d, t] += sum_fc w2[fc,d].T @ h[fc, t]    (d on PSUM partitions,
         weights stationary, h moving — in two D-half passes of 4 chunks
         so ph(2) + po(4) = 6 PSUM banks)
  Weights stay resident in SBUF; x is streamed per strip (double-buffered);
  mm1 f-chunks of strip s+1 are interleaved between the mm2 passes of
  strip s so accumulator-retire copies (split across the Vector and
  Scalar engines) never block the PE.
"""

import time

import numpy as np
import ml_dtypes

import concourse.bass as bass
import concourse.mybir as mybir
import concourse.tile as tile
from concourse import bacc
from concourse.bass_utils import run_bass_kernel_spmd

N, D, F, E, TOPK = 8192, 1024, 4096, 8, 2
P = 128
NCORES = 8
FS = F // NCORES   # 512: f-slice width per core
FC = FS // P       # 4 local f-chunks of 128
KD = D // P        # 8 k-tiles over d_model
DT = D // P        # 8 output d-chunks of 128
TS = 512           # max tokens per strip (PSUM: 2 ph + 4 po banks)

BF16 = mybir.dt.bfloat16
F32 = mybir.dt.float32

_program_cache: dict[tuple, "bass.Bass"] = {}
LAST_RESULTS = None    # BassKernelResults of the most recent run (for test.py)
TRACE = False          # test.py can flip this before calling kernel()


def _strips_of(loads) -> list[tuple]:
    """Flat [(expert, col_offset, n_tokens)] strip list over the dispatched
    token stream (A columns, expert-grouped)."""
    strips, off = [], 0
    for e, load in enumerate(loads):
        left = int(load)
        while left > 0:
            tb = min(TS, left)
            strips.append((e, off, tb))
            off += tb
            left -= tb
    return strips


def _build_program(loads: tuple, bench_iters: int = 1) -> "bass.Bass":
    """Partial FFN over this core's f-slice for all E experts:
    out[D, A] = concat_e (relu(x_e @ W1e[:, fs]) @ W2e[fs, :]).T
    with the token stream grouped by expert, segment lengths `loads`.

    bench_iters > 1 wraps the compute in a hardware loop (same result, run
    repeatedly) so test harnesses can measure steady-state HW time from the
    wall-clock delta between two iteration counts."""
    A = int(sum(loads))
    strips = _strips_of(loads)

    nc = bacc.Bacc("TRN2", target_bir_lowering=False, debug=False,
                   num_devices=NCORES)
    xT = nc.dram_tensor("xT", [D, A], BF16, kind="ExternalInput")
    w1 = nc.dram_tensor("w1", [E, D, FS], BF16, kind="ExternalInput")
    w2 = nc.dram_tensor("w2", [E, FS, D], BF16, kind="ExternalInput")
    out = nc.dram_tensor("out", [D, A], BF16, kind="ExternalOutput")

    xT_r = xT[:].rearrange("(ko p) n -> ko p n", p=P)
    w1_r = w1[:].rearrange("e (ko p) f -> e ko p f", p=P)
    w2_r = w2[:].rearrange("e (fc p) d -> e fc p d", p=P)
    out_r = out[:].rearrange("(dt p) n -> dt p n", p=P)

    with tile.TileContext(nc) as tc:
        with (
            tc.tile_pool(name="wpool", bufs=1) as wpool,
            tc.tile_pool(name="xpool", bufs=3) as xpool,
            tc.tile_pool(name="hpool", bufs=2) as hpool,
            tc.tile_pool(name="opool", bufs=4) as opool,
            tc.tile_pool(name="ph_pool", bufs=2, space="PSUM") as ph_pool,
            tc.tile_pool(name="po_pool", bufs=1, space="PSUM") as po_pool,
        ):
            w1_sb = wpool.tile([P, E, KD, FS], BF16, name="w1_sb")
            w2_sb = wpool.tile([P, E, FC, D], BF16, name="w2_sb")
            for e in range(E):
                for k in range(KD):
                    nc.sync.dma_start(w1_sb[:, e, k, :], w1_r[e, k])
                for c in range(FC):
                    nc.sync.dma_start(w2_sb[:, e, c, :], w2_r[e, c])

            def x_load(si):
                e, o0, tb = strips[si]
                xb = xpool.tile([P, KD, TS], BF16, name="xb", tag="xb")
                for k in range(KD):
                    nc.sync.dma_start(xb[:, k, :tb], xT_r[k][:, o0:o0 + tb])
                return xb

            def mm1(si, fc, xb, h):
                e, o0, tb = strips[si]
                ph = ph_pool.tile([P, TS], F32, name="ph", tag="ph")
                for ki in range(KD):
                    nc.tensor.matmul(
                        ph[:, :tb],
                        lhsT=w1_sb[:, e, ki, fc * P:(fc + 1) * P],
                        rhs=xb[:, ki, :tb],
                        start=(ki == 0),
                        stop=(ki == KD - 1),
                    )
                nc.scalar.activation(h[:, fc, :tb], ph[:, :tb],
                                     mybir.ActivationFunctionType.Relu)

            def mm2_pass(si, half, h):
                e, o0, tb = strips[si]
                for j in range(DT // 2):
                    dt = half * (DT // 2) + j
                    po = po_pool.tile([P, TS], F32, name=f"po_{j}",
                                      tag=f"po_{j}")
                    for fc in range(FC):
                        nc.tensor.matmul(
                            po[:, :tb],
                            lhsT=w2_sb[:, e, fc, dt * P:(dt + 1) * P],
                            rhs=h[:, fc, :tb],
                            start=(fc == 0),
                            stop=(fc == FC - 1),
                        )
                    o_sb = opool.tile([P, TS], BF16, name="o_sb", tag="o_sb")
                    # alternate retire engines so neither queue blocks the PE
                    if j % 2 == 0:
                        nc.vector.tensor_copy(o_sb[:, :tb], po[:, :tb])
                    else:
                        nc.scalar.activation(o_sb[:, :tb], po[:, :tb],
                                             mybir.ActivationFunctionType.Copy)
                    nc.sync.dma_start(out_r[dt][:, o0:o0 + tb], o_sb[:, :tb])

            def body():
                # software pipeline over strips: while strip s's h feeds mm2,
                # strip s+1's mm1 interleaves between the two mm2 passes
                n = len(strips)
                xb_cur = x_load(0)
                h_cur = hpool.tile([P, FC, TS], BF16, name="h", tag="h")
                for fc in range(FC):
                    mm1(0, fc, xb_cur, h_cur)
                for s in range(n):
                    if s + 1 < n:
                        xb_nxt = x_load(s + 1)
                        h_nxt = hpool.tile([P, FC, TS], BF16, name="h",
                                           tag="h")
                        mm1(s + 1, 0, xb_nxt, h_nxt)
                        mm1(s + 1, 1, xb_nxt, h_nxt)
                        mm2_pass(s, 0, h_cur)
                        mm1(s + 1, 2, xb_nxt, h_nxt)
                        mm1(s + 1, 3, xb_nxt, h_nxt)
                        mm2_pass(s, 1, h_cur)
                        xb_cur, h_cur = xb_nxt, h_nxt
                    else:
                        mm2_pass(s, 0, h_cur)
                        mm2_pass(s, 1, h_cur)

            if bench_iters > 1:
                with tc.For_i(0, bench_iters, 1):
                    body()
            else:
                body()
    nc.compile()
    return nc


def _gate_and_dispatch(x, w_gate):
    """Replicates the reference gating exactly (fp32): softmax + top-2."""
    logits = x.astype(np.float32) @ w_gate.astype(np.float32)        # [N, E]
    m = logits.max(-1, keepdims=True)
    p = np.exp(logits - m)
    probs = p / p.sum(-1, keepdims=True)
    # jax.lax.top_k: descending, ties broken by lower index -> stable argsort
    tk_idx = np.argsort(-probs, axis=1, kind="stable")[:, :TOPK]
    tk_vals = np.take_along_axis(probs, tk_idx, axis=1)
    tk_gates = tk_vals / (tk_vals.sum(-1, keepdims=True) + 1e-9)
    return tk_idx, tk_gates


def kernel(x, w_gate, W1, W2):
    global LAST_RESULTS
    x = np.asarray(x, dtype=np.float32)
    w_gate = np.asarray(w_gate, dtype=np.float32)
    W1 = np.asarray(W1, dtype=np.float32)
    W2 = np.asarray(W2, dtype=np.float32)
    n_tok = x.shape[0]

    tk_idx, tk_gates = _gate_and_dispatch(x, w_gate)

    # flat assignment lists, grouped by expert (stable within expert)
    eid = tk_idx.reshape(-1).astype(np.int64)          # expert of assignment
    gat = tk_gates.reshape(-1).astype(np.float32)      # gate of assignment
    tok = np.repeat(np.arange(n_tok), TOPK)            # token of assignment
    order = np.argsort(eid, kind="stable")
    tok_d, gat_d = tok[order], gat[order]
    loads = tuple(int(v) for v in np.bincount(eid, minlength=E))

    # dispatched token stream, transposed: [D, A] — identical on all cores
    xT_d = np.ascontiguousarray(x[tok_d].T).astype(ml_dtypes.bfloat16)

    # per-core inputs: the core's f-slice of every expert's weights
    in_maps = []
    for c in range(NCORES):
        fsl = slice(c * FS, (c + 1) * FS)
        in_maps.append({
            "xT": xT_d,
            "w1": np.ascontiguousarray(W1[:, :, fsl]).astype(ml_dtypes.bfloat16),
            "w2": np.ascontiguousarray(W2[:, fsl, :]).astype(ml_dtypes.bfloat16),
        })

    nc = _program_cache.get(loads)
    if nc is None:
        nc = _build_program(loads)
        _program_cache[loads] = nc

    try:
        res = run_bass_kernel_spmd(nc, in_maps, core_ids=list(range(NCORES)),
                                   trace=TRACE)
    except Exception:
        # transient NRT/device hiccups (e.g. NRT_EXEC_UNIT_UNRECOVERABLE)
        # have been observed to clear after a short wait — retry once
        time.sleep(20)
        res = run_bass_kernel_spmd(nc, in_maps, core_ids=list(range(NCORES)),
                                   trace=TRACE)
    LAST_RESULTS = res

    # combine: sum the per-core f-slice partials, then gate-weighted scatter
    o_full = np.zeros((D, len(tok_d)), np.float32)
    for c in range(NCORES):
        o_full += np.asarray(res.results[c]["out"]).astype(np.float32)

    y = np.zeros((n_tok, D), np.float32)
    off = 0
    for e in range(E):
        nk = loads[e]
        # token indices are unique within one expert (top-k experts are
        # distinct per token), so fancy-index += is safe here
        sl = slice(off, off + nk)
        y[tok_d[sl]] += gat_d[sl][:, None] * o_full[:, sl].T
        off += nk
    return y


# revision 5
# speedup vs baseline: 1.1585x; 1.1585x over previous
"""MoE layer (top-2 of 8 experts) on 8 Trainium2 NeuronCores.

Strategy (expert-parallel along the *F axis* — "global F-split"):
  * Host computes the (tiny) gating network: probs = softmax(x @ w_gate),
    top-2 experts + normalized gates per token.
  * The expert FFN decomposes along the hidden axis F:
        o = relu(x @ W1) @ W2 = sum_fslices relu(x @ W1[:, fs]) @ W2[fs, :]
    so core c is given the f-slice [c*F/8, (c+1)*F/8) of EVERY expert's
    W1/W2 (16.8 MB bf16 — same footprint as one whole expert) and computes
    the partial output of EVERY assignment over its slice.  Per-core work
    is exactly sum_e load_e * F/8 = N*K*F/8 — perfectly balanced across
    cores regardless of expert load skew, with zero dropped tokens.
  * All cores run the SAME program on the SAME dispatched-token stream
    (assignments grouped by expert); only the weight slices differ.
  * Host combines: o = sum_cores o_partial;  y[n] = sum_k gate[n,k]*o[slot].

Device kernel layout (per core, SPMD over 8 cores):
  inputs  xT [D, A]    bf16  dispatched tokens, transposed, expert-grouped
          w1 [E, D, FS] bf16  this core's f-slice of every expert's W1
          w2 [E, FS, D] bf16  this core's f-slice of every expert's W2
  output  out [D, A]   bf16  partial expert outputs (transposed; host
                             untransposes during the combine)
  Tokens are processed in strips of up to TS=512 (any remainder exact —
  no padding anywhere, both matmuls scale with the true token count):
    mm1: ph[f, t]  = sum_ki w1[ki,f].T @ xT[ki, t]   (f on PSUM partitions)
    relu -> h bf16 [f, 4fc, t]                        (scalar engine)
    mm2: po[d, t] += sum_fc w2[fc,d].T @ h[fc, t]    (d on PSUM partitions,
         weights stationary, h moving — in two D-half passes of 4 chunks
         so ph(2) + po(4) = 6 PSUM banks)
  Weights stay resident in SBUF; x is streamed per strip (double-buffered);
  mm1 f-chunks of strip s+1 are interleaved between the mm2 passes of
  strip s so accumulator-retire copies (split across the Vector and
  Scalar engines) never block the PE.
"""

import time

import numpy as np
import ml_dtypes

import concourse.bass as bass
import concourse.mybir as mybir
import concourse.tile as tile
from concourse import bacc
from concourse.bass_utils import run_bass_kernel_spmd

N, D, F, E, TOPK = 8192, 1024, 4096, 8, 2
P = 128
NCORES = 8
FS = F // NCORES   # 512: f-slice width per core
FC = FS // P       # 4 local f-chunks of 128
KD = D // P        # 8 k-tiles over d_model
DT = D // P        # 8 output d-chunks of 128
TS = 512           # max tokens per strip (PSUM: 2 ph + 4 po banks)

BF16 = mybir.dt.bfloat16
F32 = mybir.dt.float32

_program_cache: dict[tuple, "bass.Bass"] = {}
LAST_RESULTS = None    # BassKernelResults of the most recent run (for test.py)
TRACE = False          # test.py can flip this before calling kernel()


def _strips_of(loads) -> list[tuple]:
    """Flat [(expert, col_offset, n_tokens)] strip list over the dispatched
    token stream (A columns, expert-grouped)."""
    strips, off = [], 0
    for e, load in enumerate(loads):
        left = int(load)
        while left > 0:
            tb = min(TS, left)
            strips.append((e, off, tb))
            off += tb
            left -= tb
    return strips


def _build_program(loads: tuple, bench_iters: int = 1) -> "bass.Bass":
    """Partial FFN over this core's f-slice for all E experts:
    out[D, A] = concat_e (relu(x_e @ W1e[:, fs]) @ W2e[fs, :]).T
    with the token stream grouped by expert, segment lengths `loads`.

    bench_iters > 1 wraps the compute in a hardware loop (same result, run
    repeatedly) so test harnesses can measure steady-state HW time from the
    wall-clock delta between two iteration counts."""
    A = int(sum(loads))
    strips = _strips_of(loads)

    nc = bacc.Bacc("TRN2", target_bir_lowering=False, debug=False,
                   num_devices=NCORES)
    xT = nc.dram_tensor("xT", [D, A], BF16, kind="ExternalInput")
    w1 = nc.dram_tensor("w1", [E, D, FS], BF16, kind="ExternalInput")
    w2 = nc.dram_tensor("w2", [E, FS, D], BF16, kind="ExternalInput")
    out = nc.dram_tensor("out", [D, A], BF16, kind="ExternalOutput")

    xT_r = xT[:].rearrange("(ko p) n -> ko p n", p=P)
    w1_r = w1[:].rearrange("e (ko p) f -> e ko p f", p=P)
    w2_r = w2[:].rearrange("e (fc p) d -> e fc p d", p=P)
    out_r = out[:].rearrange("(dt p) n -> dt p n", p=P)

    with tile.TileContext(nc) as tc:
        with (
            tc.tile_pool(name="wpool", bufs=1) as wpool,
            tc.tile_pool(name="xpool", bufs=4) as xpool,
            tc.tile_pool(name="hpool", bufs=2) as hpool,
            tc.tile_pool(name="opool", bufs=4) as opool,
            tc.tile_pool(name="ph_pool", bufs=2, space="PSUM") as ph_pool,
            tc.tile_pool(name="po_pool", bufs=1, space="PSUM") as po_pool,
        ):
            w1_sb = wpool.tile([P, E, KD, FS], BF16, name="w1_sb")
            w2_sb = wpool.tile([P, E, FC, D], BF16, name="w2_sb")
            for e in range(E):
                for k in range(KD):
                    nc.sync.dma_start(w1_sb[:, e, k, :], w1_r[e, k])
                for c in range(FC):
                    nc.sync.dma_start(w2_sb[:, e, c, :], w2_r[e, c])

            def x_load(si):
                e, o0, tb = strips[si]
                xb = xpool.tile([P, KD, TS], BF16, name="xb", tag="xb")
                for k in range(KD):
                    nc.sync.dma_start(xb[:, k, :tb], xT_r[k][:, o0:o0 + tb])
                return xb

            def mm1(si, fc, xb, h):
                e, o0, tb = strips[si]
                ph = ph_pool.tile([P, TS], F32, name="ph", tag="ph")
                for ki in range(KD):
                    nc.tensor.matmul(
                        ph[:, :tb],
                        lhsT=w1_sb[:, e, ki, fc * P:(fc + 1) * P],
                        rhs=xb[:, ki, :tb],
                        start=(ki == 0),
                        stop=(ki == KD - 1),
                    )
                nc.scalar.activation(h[:, fc, :tb], ph[:, :tb],
                                     mybir.ActivationFunctionType.Relu)

            def mm2_pass(si, half, h):
                e, o0, tb = strips[si]
                pos = [po_pool.tile([P, TS], F32, name=f"po_{j}",
                                    tag=f"po_{j}") for j in range(DT // 2)]
                # fc-outer: accumulation round-robins across the 4 po banks —
                # consecutive PE matmuls never target the same PSUM bank
                # (same-bank back-to-back accumulation measurably stalls the
                # PE on hardware)
                for fc in range(FC):
                    for j in range(DT // 2):
                        dt = half * (DT // 2) + j
                        nc.tensor.matmul(
                            pos[j][:, :tb],
                            lhsT=w2_sb[:, e, fc, dt * P:(dt + 1) * P],
                            rhs=h[:, fc, :tb],
                            start=(fc == 0),
                            stop=(fc == FC - 1),
                        )
                for j in range(DT // 2):
                    dt = half * (DT // 2) + j
                    o_sb = opool.tile([P, TS], BF16, name="o_sb", tag="o_sb")
                    # alternate retire engines so neither queue blocks the PE
                    if j % 2 == 0:
                        nc.vector.tensor_copy(o_sb[:, :tb], pos[j][:, :tb])
                    else:
                        nc.scalar.activation(o_sb[:, :tb], pos[j][:, :tb],
                                             mybir.ActivationFunctionType.Copy)
                    nc.sync.dma_start(out_r[dt][:, o0:o0 + tb], o_sb[:, :tb])

            PREFETCH = 2   # x strips loaded ahead of use (hides DMA latency)

            def body():
                # software pipeline over strips: while strip s's h feeds mm2,
                # strip s+1's mm1 interleaves between the two mm2 passes
                n = len(strips)
                xbs = {i: x_load(i) for i in range(min(PREFETCH, n))}
                h_cur = hpool.tile([P, FC, TS], BF16, name="h", tag="h")
                for fc in range(FC):
                    mm1(0, fc, xbs[0], h_cur)
                for s in range(n):
                    if s + PREFETCH < n:
                        xbs[s + PREFETCH] = x_load(s + PREFETCH)
                    if s + 1 < n:
                        h_nxt = hpool.tile([P, FC, TS], BF16, name="h",
                                           tag="h")
                        mm1(s + 1, 0, xbs[s + 1], h_nxt)
                        mm1(s + 1, 1, xbs[s + 1], h_nxt)
                        mm2_pass(s, 0, h_cur)
                        mm1(s + 1, 2, xbs[s + 1], h_nxt)
                        mm1(s + 1, 3, xbs[s + 1], h_nxt)
                        mm2_pass(s, 1, h_cur)
                        h_cur = h_nxt
                        del xbs[s]
                    else:
                        mm2_pass(s, 0, h_cur)
                        mm2_pass(s, 1, h_cur)

            if bench_iters > 1:
                with tc.For_i(0, bench_iters, 1):
                    body()
            else:
                body()
    nc.compile()
    return nc


def _gate_and_dispatch(x, w_gate):
    """Replicates the reference gating exactly (fp32): softmax + top-2."""
    logits = x.astype(np.float32) @ w_gate.astype(np.float32)        # [N, E]
    m = logits.max(-1, keepdims=True)
    p = np.exp(logits - m)
    probs = p / p.sum(-1, keepdims=True)
    # jax.lax.top_k: descending, ties broken by lower index -> stable argsort
    tk_idx = np.argsort(-probs, axis=1, kind="stable")[:, :TOPK]
    tk_vals = np.take_along_axis(probs, tk_idx, axis=1)
    tk_gates = tk_vals / (tk_vals.sum(-1, keepdims=True) + 1e-9)
    return tk_idx, tk_gates


def kernel(x, w_gate, W1, W2):
    global LAST_RESULTS
    x = np.asarray(x, dtype=np.float32)
    w_gate = np.asarray(w_gate, dtype=np.float32)
    W1 = np.asarray(W1, dtype=np.float32)
    W2 = np.asarray(W2, dtype=np.float32)
    n_tok = x.shape[0]

    tk_idx, tk_gates = _gate_and_dispatch(x, w_gate)

    # flat assignment lists, grouped by expert (stable within expert)
    eid = tk_idx.reshape(-1).astype(np.int64)          # expert of assignment
    gat = tk_gates.reshape(-1).astype(np.float32)      # gate of assignment
    tok = np.repeat(np.arange(n_tok), TOPK)            # token of assignment
    order = np.argsort(eid, kind="stable")
    tok_d, gat_d = tok[order], gat[order]
    loads = tuple(int(v) for v in np.bincount(eid, minlength=E))

    # dispatched token stream, transposed: [D, A] — identical on all cores
    xT_d = np.ascontiguousarray(x[tok_d].T).astype(ml_dtypes.bfloat16)

    # per-core inputs: the core's f-slice of every expert's weights
    in_maps = []
    for c in range(NCORES):
        fsl = slice(c * FS, (c + 1) * FS)
        in_maps.append({
            "xT": xT_d,
            "w1": np.ascontiguousarray(W1[:, :, fsl]).astype(ml_dtypes.bfloat16),
            "w2": np.ascontiguousarray(W2[:, fsl, :]).astype(ml_dtypes.bfloat16),
        })

    nc = _program_cache.get(loads)
    if nc is None:
        nc = _build_program(loads)
        _program_cache[loads] = nc

    try:
        res = run_bass_kernel_spmd(nc, in_maps, core_ids=list(range(NCORES)),
                                   trace=TRACE)
    except Exception:
        # transient NRT/device hiccups (e.g. NRT_EXEC_UNIT_UNRECOVERABLE)
        # have been observed to clear after a short wait — retry once
        time.sleep(20)
        res = run_bass_kernel_spmd(nc, in_maps, core_ids=list(range(NCORES)),
                                   trace=TRACE)
    LAST_RESULTS = res

    # combine: sum the per-core f-slice partials, then gate-weighted scatter
    o_full = np.zeros((D, len(tok_d)), np.float32)
    for c in range(NCORES):
        o_full += np.asarray(res.results[c]["out"]).astype(np.float32)

    y = np.zeros((n_tok, D), np.float32)
    off = 0
    for e in range(E):
        nk = loads[e]
        # token indices are unique within one expert (top-k experts are
        # distinct per token), so fancy-index += is safe here
        sl = slice(off, off + nk)
        y[tok_d[sl]] += gat_d[sl][:, None] * o_full[:, sl].T
        off += nk
    return y


# revision 7
# speedup vs baseline: 1.1964x; 1.0327x over previous
"""MoE layer (top-2 of 8 experts) on 8 Trainium2 NeuronCores.

Strategy (expert-parallel along the *F axis* — "global F-split"):
  * Host computes the (tiny) gating network: probs = softmax(x @ w_gate),
    top-2 experts + normalized gates per token.
  * The expert FFN decomposes along the hidden axis F:
        o = relu(x @ W1) @ W2 = sum_fslices relu(x @ W1[:, fs]) @ W2[fs, :]
    so core c is given the f-slice [c*F/8, (c+1)*F/8) of EVERY expert's
    W1/W2 (16.8 MB bf16 — same footprint as one whole expert) and computes
    the partial output of EVERY assignment over its slice.  Per-core work
    is exactly sum_e load_e * F/8 = N*K*F/8 — perfectly balanced across
    cores regardless of expert load skew, with zero dropped tokens.
  * All cores run the SAME program on the SAME dispatched-token stream
    (assignments grouped by expert); only the weight slices differ.
  * Host combines: o = sum_cores o_partial;  y[n] = sum_k gate[n,k]*o[slot].

Device kernel layout (per core, SPMD over 8 cores):
  inputs  xT [D, A]    bf16  dispatched tokens, transposed, expert-grouped
          w1 [E, D, FS] bf16  this core's f-slice of every expert's W1
          w2 [E, FS, D] bf16  this core's f-slice of every expert's W2
  output  out [D, A]   bf16  partial expert outputs (transposed; host
                             untransposes during the combine)
  Tokens are processed in strips of up to TS=512 (any remainder exact —
  no padding anywhere, both matmuls scale with the true token count):
    mm1: ph[f, t]  = sum_ki w1[ki,f].T @ xT[ki, t]   (f on PSUM partitions)
    relu -> h bf16 [f, 4fc, t]                        (scalar engine)
    mm2: po[d, t] += sum_fc w2[fc,d].T @ h[fc, t]    (d on PSUM partitions,
         weights stationary, h moving — in two D-half passes of 4 chunks
         so ph(2) + po(4) = 6 PSUM banks)
  Weights stay resident in SBUF; x is streamed per strip (double-buffered);
  mm1 f-chunks of strip s+1 are interleaved between the mm2 passes of
  strip s so accumulator-retire copies (split across the Vector and
  Scalar engines) never block the PE.
"""

import time

import numpy as np
import ml_dtypes

import concourse.bass as bass
import concourse.mybir as mybir
import concourse.tile as tile
from concourse import bacc
from concourse.bass_utils import run_bass_kernel_spmd

N, D, F, E, TOPK = 8192, 1024, 4096, 8, 2
P = 128
NCORES = 8
FS = F // NCORES   # 512: f-slice width per core
FC = FS // P       # 4 local f-chunks of 128
KD = D // P        # 8 k-tiles over d_model
DT = D // P        # 8 output d-chunks of 128
TS = 512           # max tokens per strip (PSUM: 2 ph + 4 po banks)

BF16 = mybir.dt.bfloat16
F32 = mybir.dt.float32

_program_cache: dict[tuple, "bass.Bass"] = {}
LAST_RESULTS = None    # BassKernelResults of the most recent run (for test.py)
TRACE = False          # test.py can flip this before calling kernel()


def _strips_of(loads) -> list[tuple]:
    """Flat [(expert, col_offset, n_tokens)] strip list over the dispatched
    token stream (A columns, expert-grouped)."""
    strips, off = [], 0
    for e, load in enumerate(loads):
        left = int(load)
        while left > 0:
            tb = min(TS, left)
            strips.append((e, off, tb))
            off += tb
            left -= tb
    return strips


def _build_program(loads: tuple, bench_iters: int = 1) -> "bass.Bass":
    """Partial FFN over this core's f-slice for all E experts:
    out[D, A] = concat_e (relu(x_e @ W1e[:, fs]) @ W2e[fs, :]).T
    with the token stream grouped by expert, segment lengths `loads`.

    bench_iters > 1 wraps the compute in a hardware loop (same result, run
    repeatedly) so test harnesses can measure steady-state HW time from the
    wall-clock delta between two iteration counts."""
    A = int(sum(loads))
    strips = _strips_of(loads)

    nc = bacc.Bacc("TRN2", target_bir_lowering=False, debug=False,
                   num_devices=NCORES)
    xT = nc.dram_tensor("xT", [D, A], BF16, kind="ExternalInput")
    w1 = nc.dram_tensor("w1", [E, D, FS], BF16, kind="ExternalInput")
    w2 = nc.dram_tensor("w2", [E, FS, D], BF16, kind="ExternalInput")
    out = nc.dram_tensor("out", [D, A], BF16, kind="ExternalOutput")

    xT_r = xT[:].rearrange("(ko p) n -> ko p n", p=P)
    w1_r = w1[:].rearrange("e (ko p) f -> e ko p f", p=P)
    w2_r = w2[:].rearrange("e (fc p) d -> e fc p d", p=P)
    out_r = out[:].rearrange("(dt p) n -> dt p n", p=P)

    with tile.TileContext(nc) as tc:
        with (
            tc.tile_pool(name="wpool", bufs=1) as wpool,
            tc.tile_pool(name="xpool", bufs=4) as xpool,
            tc.tile_pool(name="hpool", bufs=2) as hpool,
            tc.tile_pool(name="opool", bufs=4) as opool,
            tc.tile_pool(name="ph_pool", bufs=2, space="PSUM") as ph_pool,
            tc.tile_pool(name="po_pool", bufs=1, space="PSUM") as po_pool,
        ):
            w1_sb = wpool.tile([P, E, KD, FS], BF16, name="w1_sb")
            w2_sb = wpool.tile([P, E, FC, D], BF16, name="w2_sb")
            for e in range(E):
                for k in range(KD):
                    nc.sync.dma_start(w1_sb[:, e, k, :], w1_r[e, k])
                for c in range(FC):
                    nc.sync.dma_start(w2_sb[:, e, c, :], w2_r[e, c])

            def x_load(si):
                e, o0, tb = strips[si]
                xb = xpool.tile([P, KD, TS], BF16, name="xb", tag="xb")
                for k in range(KD):
                    nc.sync.dma_start(xb[:, k, :tb], xT_r[k][:, o0:o0 + tb])
                return xb

            def mm1(si, fc, xb, h):
                e, o0, tb = strips[si]
                ph = ph_pool.tile([P, TS], F32, name="ph", tag="ph")
                for ki in range(KD):
                    nc.tensor.matmul(
                        ph[:, :tb],
                        lhsT=w1_sb[:, e, ki, fc * P:(fc + 1) * P],
                        rhs=xb[:, ki, :tb],
                        start=(ki == 0),
                        stop=(ki == KD - 1),
                    )
                nc.scalar.activation(h[:, fc, :tb], ph[:, :tb],
                                     mybir.ActivationFunctionType.Relu)

            def mm2_pass(si, half, h):
                e, o0, tb = strips[si]
                pos = [po_pool.tile([P, TS], F32, name=f"po_{j}",
                                    tag=f"po_{j}") for j in range(DT // 2)]
                # fc-outer: accumulation round-robins across the 4 po banks —
                # consecutive PE matmuls never target the same PSUM bank
                # (same-bank back-to-back accumulation measurably stalls the
                # PE on hardware)
                for fc in range(FC):
                    for j in range(DT // 2):
                        dt = half * (DT // 2) + j
                        nc.tensor.matmul(
                            pos[j][:, :tb],
                            lhsT=w2_sb[:, e, fc, dt * P:(dt + 1) * P],
                            rhs=h[:, fc, :tb],
                            start=(fc == 0),
                            stop=(fc == FC - 1),
                        )
                for j in range(DT // 2):
                    dt = half * (DT // 2) + j
                    o_sb = opool.tile([P, TS], BF16, name="o_sb", tag="o_sb")
                    # alternate retire engines so neither queue blocks the PE
                    if j % 2 == 0:
                        nc.vector.tensor_copy(o_sb[:, :tb], pos[j][:, :tb])
                    else:
                        nc.scalar.activation(o_sb[:, :tb], pos[j][:, :tb],
                                             mybir.ActivationFunctionType.Copy)
                    nc.sync.dma_start(out_r[dt][:, o0:o0 + tb], o_sb[:, :tb])

            PREFETCH = 2   # x strips loaded ahead of use (hides DMA latency)
            n = len(strips)

            def body():
                # software pipeline over strips: while strip s's h feeds mm2,
                # strip (s+1) mod n's mm1 interleaves between the two mm2
                # passes.  The pipeline is *circular*: the body's tail mm1
                # computes strip 0's h for the NEXT For_i iteration (the
                # prologue before the loop primes it once), so the hardware
                # loop has no un-overlapped prologue/epilogue in steady state.
                # With bench_iters == 1 the tail mm1 is recomputed dead work
                # in both the 1-iter and M-iter programs, so it cancels in
                # the (wall_M - wall_1) / (M - 1) steady-state estimate.
                nonlocal h_cur
                for s in range(n):
                    if n > 1:
                        xbs[s + PREFETCH] = x_load((s + PREFETCH) % n)
                        h_nxt = hpool.tile([P, FC, TS], BF16, name="h",
                                           tag="h")
                        sn = (s + 1) % n
                        mm1(sn, 0, xbs[s + 1], h_nxt)
                        mm1(sn, 1, xbs[s + 1], h_nxt)
                        mm2_pass(s, 0, h_cur)
                        mm1(sn, 2, xbs[s + 1], h_nxt)
                        mm1(sn, 3, xbs[s + 1], h_nxt)
                        mm2_pass(s, 1, h_cur)
                        h_cur = h_nxt
                        del xbs[s]
                    else:
                        mm2_pass(s, 0, h_cur)
                        mm2_pass(s, 1, h_cur)

            # prologue: prime x and strip-0's h once, outside the loop
            xbs = {i: x_load(i % n) for i in range(min(PREFETCH, n))}
            h_cur = hpool.tile([P, FC, TS], BF16, name="h", tag="h")
            for fc in range(FC):
                mm1(0, fc, xbs[0], h_cur)
            if bench_iters > 1:
                with tc.For_i(0, bench_iters, 1):
                    body()
            else:
                body()
    nc.compile()
    return nc


def _gate_and_dispatch(x, w_gate):
    """Replicates the reference gating exactly (fp32): softmax + top-2."""
    logits = x.astype(np.float32) @ w_gate.astype(np.float32)        # [N, E]
    m = logits.max(-1, keepdims=True)
    p = np.exp(logits - m)
    probs = p / p.sum(-1, keepdims=True)
    # jax.lax.top_k: descending, ties broken by lower index -> stable argsort
    tk_idx = np.argsort(-probs, axis=1, kind="stable")[:, :TOPK]
    tk_vals = np.take_along_axis(probs, tk_idx, axis=1)
    tk_gates = tk_vals / (tk_vals.sum(-1, keepdims=True) + 1e-9)
    return tk_idx, tk_gates


def kernel(x, w_gate, W1, W2):
    global LAST_RESULTS
    x = np.asarray(x, dtype=np.float32)
    w_gate = np.asarray(w_gate, dtype=np.float32)
    W1 = np.asarray(W1, dtype=np.float32)
    W2 = np.asarray(W2, dtype=np.float32)
    n_tok = x.shape[0]

    tk_idx, tk_gates = _gate_and_dispatch(x, w_gate)

    # flat assignment lists, grouped by expert (stable within expert)
    eid = tk_idx.reshape(-1).astype(np.int64)          # expert of assignment
    gat = tk_gates.reshape(-1).astype(np.float32)      # gate of assignment
    tok = np.repeat(np.arange(n_tok), TOPK)            # token of assignment
    order = np.argsort(eid, kind="stable")
    tok_d, gat_d = tok[order], gat[order]
    loads = tuple(int(v) for v in np.bincount(eid, minlength=E))

    # dispatched token stream, transposed: [D, A] — identical on all cores
    xT_d = np.ascontiguousarray(x[tok_d].T).astype(ml_dtypes.bfloat16)

    # per-core inputs: the core's f-slice of every expert's weights
    in_maps = []
    for c in range(NCORES):
        fsl = slice(c * FS, (c + 1) * FS)
        in_maps.append({
            "xT": xT_d,
            "w1": np.ascontiguousarray(W1[:, :, fsl]).astype(ml_dtypes.bfloat16),
            "w2": np.ascontiguousarray(W2[:, fsl, :]).astype(ml_dtypes.bfloat16),
        })

    nc = _program_cache.get(loads)
    if nc is None:
        nc = _build_program(loads)
        _program_cache[loads] = nc

    # transient NRT/device hiccups (e.g. NRT_EXEC_UNIT_UNRECOVERABLE) have
    # been observed to clear after a short wait — retry with backoff
    res = None
    for attempt, pause in enumerate((0, 20, 60, 120)):
        if pause:
            time.sleep(pause)
        try:
            res = run_bass_kernel_spmd(nc, in_maps,
                                       core_ids=list(range(NCORES)),
                                       trace=TRACE)
            break
        except Exception:
            if attempt == 3:
                raise
    LAST_RESULTS = res

    # combine: sum the per-core f-slice partials, then gate-weighted scatter
    o_full = np.zeros((D, len(tok_d)), np.float32)
    for c in range(NCORES):
        o_full += np.asarray(res.results[c]["out"]).astype(np.float32)

    y = np.zeros((n_tok, D), np.float32)
    off = 0
    for e in range(E):
        nk = loads[e]
        # token indices are unique within one expert (top-k experts are
        # distinct per token), so fancy-index += is safe here
        sl = slice(off, off + nk)
        y[tok_d[sl]] += gat_d[sl][:, None] * o_full[:, sl].T
        off += nk
    return y


# revision 8
# speedup vs baseline: 1.3175x; 1.1012x over previous
"""MoE layer (top-2 of 8 experts) on 8 Trainium2 NeuronCores.

Strategy (expert-parallel along the *F axis* — "global F-split"):
  * Host computes the (tiny) gating network: probs = softmax(x @ w_gate),
    top-2 experts + normalized gates per token.
  * The expert FFN decomposes along the hidden axis F:
        o = relu(x @ W1) @ W2 = sum_fslices relu(x @ W1[:, fs]) @ W2[fs, :]
    so core c is given the f-slice [c*F/8, (c+1)*F/8) of EVERY expert's
    W1/W2 (16.8 MB bf16 — same footprint as one whole expert) and computes
    the partial output of EVERY assignment over its slice.  Per-core work
    is exactly sum_e load_e * F/8 = N*K*F/8 — perfectly balanced across
    cores regardless of expert load skew, with zero dropped tokens.
  * All cores run the SAME program on the SAME dispatched-token stream
    (assignments grouped by expert); only the weight slices differ.
  * Host combines: o = sum_cores o_partial;  y[n] = sum_k gate[n,k]*o[slot].

Device kernel layout (per core, SPMD over 8 cores):
  inputs  xT [D, A]    bf16  dispatched tokens, transposed, expert-grouped
          w1 [E, D, FS] bf16  this core's f-slice of every expert's W1
          w2 [E, FS, D] bf16  this core's f-slice of every expert's W2
  output  out [D, A]   bf16  partial expert outputs (transposed; host
                             untransposes during the combine)
  Tokens are processed in strips of up to TS=512 (any remainder exact —
  no padding anywhere, both matmuls scale with the true token count):
    mm1: ph[f, t]  = sum_ki w1[ki,f].T @ xT[ki, t]   (f on PSUM partitions)
    relu -> h bf16 [f, 4fc, t]                        (scalar engine)
    mm2: po[d, t] += sum_fc w2[fc,d].T @ h[fc, t]    (d on PSUM partitions,
         weights stationary, h moving — in two D-half passes of 4 chunks
         so ph(2) + po(4) = 6 PSUM banks)
  Weights stay resident in SBUF; x is streamed per strip (double-buffered);
  mm1 f-chunks of strip s+1 are interleaved between the mm2 passes of
  strip s so accumulator-retire copies (split across the Vector and
  Scalar engines) never block the PE.
"""

import time

import numpy as np
import ml_dtypes

import concourse.bass as bass
import concourse.mybir as mybir
import concourse.tile as tile
from concourse import bacc
from concourse.bass_utils import run_bass_kernel_spmd

N, D, F, E, TOPK = 8192, 1024, 4096, 8, 2
P = 128
NCORES = 8
FS = F // NCORES   # 512: f-slice width per core
FC = FS // P       # 4 local f-chunks of 128
KD = D // P        # 8 k-tiles over d_model
DT = D // P        # 8 output d-chunks of 128
TS = 512           # max tokens per strip (PSUM: 2 ph + 4 po banks)

BF16 = mybir.dt.bfloat16
F32 = mybir.dt.float32

_program_cache: dict[tuple, "bass.Bass"] = {}
LAST_RESULTS = None    # BassKernelResults of the most recent run (for test.py)
TRACE = False          # test.py can flip this before calling kernel()


def _strips_of(loads) -> list[tuple]:
    """Flat [(expert, col_offset, n_tokens)] strip list over the dispatched
    token stream (A columns, expert-grouped)."""
    strips, off = [], 0
    for e, load in enumerate(loads):
        left = int(load)
        while left > 0:
            tb = min(TS, left)
            strips.append((e, off, tb))
            off += tb
            left -= tb
    return strips


def _build_program(loads: tuple, bench_iters: int = 1) -> "bass.Bass":
    """Partial FFN over this core's f-slice for all E experts:
    out[D, A] = concat_e (relu(x_e @ W1e[:, fs]) @ W2e[fs, :]).T
    with the token stream grouped by expert, segment lengths `loads`.

    bench_iters > 1 wraps the compute in a hardware loop (same result, run
    repeatedly) so test harnesses can measure steady-state HW time from the
    wall-clock delta between two iteration counts."""
    A = int(sum(loads))
    strips = _strips_of(loads)

    nc = bacc.Bacc("TRN2", target_bir_lowering=False, debug=False,
                   num_devices=NCORES)
    xT = nc.dram_tensor("xT", [D, A], BF16, kind="ExternalInput")
    w1 = nc.dram_tensor("w1", [E, D, FS], BF16, kind="ExternalInput")
    w2 = nc.dram_tensor("w2", [E, FS, D], BF16, kind="ExternalInput")
    out = nc.dram_tensor("out", [D, A], BF16, kind="ExternalOutput")

    xT_r = xT[:].rearrange("(ko p) n -> ko p n", p=P)
    w1_r = w1[:].rearrange("e (ko p) f -> e ko p f", p=P)
    w2_r = w2[:].rearrange("e (fc p) d -> e fc p d", p=P)
    out_r = out[:].rearrange("(dt p) n -> dt p n", p=P)

    with tile.TileContext(nc) as tc:
        with (
            tc.tile_pool(name="wpool", bufs=1) as wpool,
            tc.tile_pool(name="xpool", bufs=4) as xpool,
            tc.tile_pool(name="hpool", bufs=2) as hpool,
            tc.tile_pool(name="opool", bufs=4) as opool,
            tc.tile_pool(name="ph_pool", bufs=2, space="PSUM") as ph_pool,
            tc.tile_pool(name="po_pool", bufs=1, space="PSUM") as po_pool,
        ):
            w1_sb = wpool.tile([P, E, KD, FS], BF16, name="w1_sb")
            w2_sb = wpool.tile([P, E, FC, D], BF16, name="w2_sb")
            for e in range(E):
                for k in range(KD):
                    nc.sync.dma_start(w1_sb[:, e, k, :], w1_r[e, k])
                for c in range(FC):
                    nc.sync.dma_start(w2_sb[:, e, c, :], w2_r[e, c])

            def x_load(si):
                e, o0, tb = strips[si]
                xb = xpool.tile([P, KD, TS], BF16, name="xb", tag="xb")
                for k in range(KD):
                    nc.sync.dma_start(xb[:, k, :tb], xT_r[k][:, o0:o0 + tb])
                return xb

            def mm1(si, fc, xb, h):
                e, o0, tb = strips[si]
                ph = ph_pool.tile([P, TS], F32, name="ph", tag="ph")
                for ki in range(KD):
                    nc.tensor.matmul(
                        ph[:, :tb],
                        lhsT=w1_sb[:, e, ki, fc * P:(fc + 1) * P],
                        rhs=xb[:, ki, :tb],
                        start=(ki == 0),
                        stop=(ki == KD - 1),
                    )
                nc.scalar.activation(h[:, fc, :tb], ph[:, :tb],
                                     mybir.ActivationFunctionType.Relu)

            def mm2_pass(si, half, h):
                e, o0, tb = strips[si]
                pos = [po_pool.tile([P, TS], F32, name=f"po_{j}",
                                    tag=f"po_{j}") for j in range(DT // 2)]
                # fc-outer: accumulation round-robins across the 4 po banks —
                # consecutive PE matmuls never target the same PSUM bank
                # (same-bank back-to-back accumulation measurably stalls the
                # PE on hardware)
                for fc in range(FC):
                    for j in range(DT // 2):
                        dt = half * (DT // 2) + j
                        nc.tensor.matmul(
                            pos[j][:, :tb],
                            lhsT=w2_sb[:, e, fc, dt * P:(dt + 1) * P],
                            rhs=h[:, fc, :tb],
                            start=(fc == 0),
                            stop=(fc == FC - 1),
                        )
                for j in range(DT // 2):
                    dt = half * (DT // 2) + j
                    o_sb = opool.tile([P, TS], BF16, name="o_sb", tag="o_sb")
                    # alternate retire engines so neither queue blocks the PE
                    if j % 2 == 0:
                        nc.vector.tensor_copy(o_sb[:, :tb], pos[j][:, :tb])
                    else:
                        nc.scalar.activation(o_sb[:, :tb], pos[j][:, :tb],
                                             mybir.ActivationFunctionType.Copy)
                    nc.sync.dma_start(out_r[dt][:, o0:o0 + tb], o_sb[:, :tb])

            PREFETCH = 2   # x strips loaded ahead of use (hides DMA latency)

            def body():
                # software pipeline over strips: while strip s's h feeds mm2,
                # strip s+1's mm1 interleaves between the two mm2 passes
                n = len(strips)
                xbs = {i: x_load(i) for i in range(min(PREFETCH, n))}
                h_cur = hpool.tile([P, FC, TS], BF16, name="h", tag="h")
                for fc in range(FC):
                    mm1(0, fc, xbs[0], h_cur)
                for s in range(n):
                    if s + PREFETCH < n:
                        xbs[s + PREFETCH] = x_load(s + PREFETCH)
                    if s + 1 < n:
                        h_nxt = hpool.tile([P, FC, TS], BF16, name="h",
                                           tag="h")
                        mm1(s + 1, 0, xbs[s + 1], h_nxt)
                        mm1(s + 1, 1, xbs[s + 1], h_nxt)
                        mm2_pass(s, 0, h_cur)
                        mm1(s + 1, 2, xbs[s + 1], h_nxt)
                        mm1(s + 1, 3, xbs[s + 1], h_nxt)
                        mm2_pass(s, 1, h_cur)
                        h_cur = h_nxt
                        del xbs[s]
                    else:
                        mm2_pass(s, 0, h_cur)
                        mm2_pass(s, 1, h_cur)

            if bench_iters > 1:
                with tc.For_i(0, bench_iters, 1):
                    body()
            else:
                body()
    nc.compile()
    return nc


def _gate_and_dispatch(x, w_gate):
    """Replicates the reference gating exactly (fp32): softmax + top-2."""
    logits = x.astype(np.float32) @ w_gate.astype(np.float32)        # [N, E]
    m = logits.max(-1, keepdims=True)
    p = np.exp(logits - m)
    probs = p / p.sum(-1, keepdims=True)
    # jax.lax.top_k: descending, ties broken by lower index -> stable argsort
    tk_idx = np.argsort(-probs, axis=1, kind="stable")[:, :TOPK]
    tk_vals = np.take_along_axis(probs, tk_idx, axis=1)
    tk_gates = tk_vals / (tk_vals.sum(-1, keepdims=True) + 1e-9)
    return tk_idx, tk_gates


def kernel(x, w_gate, W1, W2):
    global LAST_RESULTS
    x = np.asarray(x, dtype=np.float32)
    w_gate = np.asarray(w_gate, dtype=np.float32)
    W1 = np.asarray(W1, dtype=np.float32)
    W2 = np.asarray(W2, dtype=np.float32)
    n_tok = x.shape[0]

    tk_idx, tk_gates = _gate_and_dispatch(x, w_gate)

    # flat assignment lists, grouped by expert (stable within expert)
    eid = tk_idx.reshape(-1).astype(np.int64)          # expert of assignment
    gat = tk_gates.reshape(-1).astype(np.float32)      # gate of assignment
    tok = np.repeat(np.arange(n_tok), TOPK)            # token of assignment
    order = np.argsort(eid, kind="stable")
    tok_d, gat_d = tok[order], gat[order]
    loads = tuple(int(v) for v in np.bincount(eid, minlength=E))

    # dispatched token stream, transposed: [D, A] — identical on all cores
    xT_d = np.ascontiguousarray(x[tok_d].T).astype(ml_dtypes.bfloat16)

    # per-core inputs: the core's f-slice of every expert's weights
    in_maps = []
    for c in range(NCORES):
        fsl = slice(c * FS, (c + 1) * FS)
        in_maps.append({
            "xT": xT_d,
            "w1": np.ascontiguousarray(W1[:, :, fsl]).astype(ml_dtypes.bfloat16),
            "w2": np.ascontiguousarray(W2[:, fsl, :]).astype(ml_dtypes.bfloat16),
        })

    nc = _program_cache.get(loads)
    if nc is None:
        nc = _build_program(loads)
        _program_cache[loads] = nc

    # transient NRT/device hiccups (e.g. NRT_EXEC_UNIT_UNRECOVERABLE) have
    # been observed to clear after a short wait — retry with backoff
    res = None
    for attempt, pause in enumerate((0, 20, 60, 120)):
        if pause:
            time.sleep(pause)
        try:
            res = run_bass_kernel_spmd(nc, in_maps,
                                       core_ids=list(range(NCORES)),
                                       trace=TRACE)
            break
        except Exception:
            if attempt == 3:
                raise
    LAST_RESULTS = res

    # combine: sum the per-core f-slice partials, then gate-weighted scatter
    o_full = np.zeros((D, len(tok_d)), np.float32)
    for c in range(NCORES):
        o_full += np.asarray(res.results[c]["out"]).astype(np.float32)

    y = np.zeros((n_tok, D), np.float32)
    off = 0
    for e in range(E):
        nk = loads[e]
        # token indices are unique within one expert (top-k experts are
        # distinct per token), so fancy-index += is safe here
        sl = slice(off, off + nk)
        y[tok_d[sl]] += gat_d[sl][:, None] * o_full[:, sl].T
        off += nk
    return y
